# revision 22
# baseline (speedup 1.0000x reference)
"""Trainium2 Bass kernel for nn_CriterionLP (LP contrastive criterion loss).

Reference computation (B=2048 anchors, M=16384 supports, C=256, K=128 label
groups of G=128 supports each):
    sim   = (feats @ Fs.T) / TEMP                  [B, M]
    E     = exp(sim) grouped into K blocks of G    [B, K, G]
    pos   = exp(min sim over own-label block)      (one block per row)
    neg   = sum over other blocks of exp(max sim over block)
    loss  = mean_b( -log(pos/(pos+neg+eps) + eps) )

v22 design (support-sharded, 8 cores, no on-device collective):
  - fp8 e4m3 DoubleRow matmuls: full C=256 contraction per instruction,
    ~250-400ns per [128, 512] output; half the input DMA of bf16.
    fp8 end-to-end rel err ~6e-4 against a 2e-2 tolerance.
  - work unit = half b-tile [128, 1024] PSUM (2 banks, 4-deep ring).
  - TRN2 engine rules: gpsimd has no PSUM access and no max; Act has no
    max; DVE reads at most ONE PSUM operand per op. Split:
      D-units (8, b-tiles 0-3): one DVE tensor_reduce straight off PSUM
              -> final [128, 8] block maxes (W=1). Three D-units are
              placed at the very end so the kernel tail is a cheap
              reduce instead of copy+TT-chain+DMA.
      A-units (24, b-tiles 4-15): Act copies the [128, 1024] tile to
              bf16 SBUF; DVE TT-maxes the block halves at 2x, then one
              more per-unit 2x TT level down to W=32; the HOST finishes
              max-of-32. (Per-unit beats cell-batched chains: no
              cell-completion sync, better overlap. W=64 direct-ship was
              ~7us slower: 3MB of output DMA interferes mid-kernel.)
    Balances Act (~27us) and DVE (~27us); PE (~16-26us) hides under.
  - min stats are needed only for own-label blocks; after the row
    rotation they live in D-units (0,h0) and (1,h1): two extra DVE
    min-reduces off the same PSUM.
  - first input chunks are split small (128KB) so the first matmul can
    start as soon as possible after the ~8.6us DMA queue bring-up.
  - outputs: stat [128,4,16] (W=1 rows 0-3), statw [128,2,12,8,32]
    (h-major so each chunk is contiguous; DMA'd per cell as it
    finishes), mins [128,2,8]. The HOST does exp, label masks, the
    cross-core sum and the -log mean (the gather/unshard step). No
    AllReduce, no cross-core barrier, cores fully decoupled.
"""

import numpy as np
import ml_dtypes

import concourse.bass as bass
import concourse.bacc as bacc
import concourse.tile as tile
import concourse.mybir as mybir
from concourse.bass_utils import run_bass_kernel_spmd

VERSION_TAG = "v22"

F32 = mybir.dt.float32
BF16 = mybir.dt.bfloat16
F8 = mybir.dt.float8e4
AX = mybir.AxisListType
ALU = mybir.AluOpType
DRMODE = mybir.MatmulPerfMode.DoubleRow

TEMP = 0.05
EPS = 1e-6
SCALE = 16.0                # fp8 quantization scale (scores come out *S^2)
B, C = 2048, 256
NCORES = 8
KTOT, G = 128, 128          # label groups, supports per group
MLOC = 2048                 # support rows per core
KLOC = KTOT // NCORES       # groups per core (16)
NBT = B // 128              # b tiles of 128 rows (16)
W = 32                      # shipped stat width for A rows
NA2B = NBT - 4              # A b-tiles (4..15)

# A chain cells: (h, first bt, n bts)
CELLS = [(0, 4, 8), (0, 12, 4), (1, 4, 8), (1, 12, 4)]

_PROG_CACHE = {}
LAST_RESULT = None          # BassKernelResults of the most recent run


def _units():
    """32 (kind, (bt, h)) units. D spread early + 3 at the end (cheap
    tail); min-bearing D-units (0,h0) and (1,h1) first."""
    d = [(0, 0), (1, 1), (0, 1), (1, 0), (2, 0), (2, 1), (3, 0), (3, 1)]
    a = [(bt, 0) for bt in range(4, 16)] + [(bt, 1) for bt in range(4, 16)]
    d_slots = {0, 4, 8, 12, 16, 20, 24, 28}
    order = []
    di, ai = 0, 0
    for i in range(32):
        if i in d_slots:
            order.append(("D", d[di])); di += 1
        else:
            order.append(("A", a[ai])); ai += 1
    return order


def _chain(nc, pool, src, out_ap, dims, w0, w1, op, tag):
    """[128, *dims, w0] bf16 -> [128, *dims, w1] via TT halving (2x)."""
    cur, w = src, w0
    while w > w1:
        hw = w // 2
        if hw == w1:
            nxt_ap = out_ap
        else:
            nxt = pool.tile([128, *dims, hw], BF16, name=f"c{hw}{tag}",
                            tag=f"c{hw}_{'_'.join(map(str, dims))}", bufs=2)
            nxt_ap = nxt[:]
        sel0 = (slice(None),) * (1 + len(dims)) + (slice(0, hw),)
        sel1 = (slice(None),) * (1 + len(dims)) + (slice(hw, w),)
        nc.vector.tensor_tensor(nxt_ap, cur[sel0], cur[sel1], op)
        if hw > w1:
            cur = nxt
        w = hw


def _build(fast):
    if fast in _PROG_CACHE:
        return _PROG_CACHE[fast]

    nc = bacc.Bacc("TRN2", target_bir_lowering=False, debug=False,
                   num_devices=NCORES)
    # small first chunks so the first unit's inputs land early
    ft0ad = nc.dram_tensor("ftq0a", [128, 2, 512], F8, kind="ExternalInput")
    ft0bd = nc.dram_tensor("ftq0b", [128, 2, 512], F8, kind="ExternalInput")
    ft1d = nc.dram_tensor("ftq1", [128, 2, 1024], F8, kind="ExternalInput")
    fs0ad = nc.dram_tensor("fsq0a", [128, 2, 512], F8, kind="ExternalInput")
    fs0bd = nc.dram_tensor("fsq0b", [128, 2, 512], F8, kind="ExternalInput")
    fs1d = nc.dram_tensor("fsq1", [128, 2, 1024], F8, kind="ExternalInput")
    statd = nc.dram_tensor("stat", [128, 4, KLOC], BF16,
                           kind="ExternalOutput")
    statwd = nc.dram_tensor("statw", [128, 2, NA2B, 8, W], BF16,
                            kind="ExternalOutput")
    mind = nc.dram_tensor("mins", [128, 2, 8] if fast else [128, 4, KLOC],
                          BF16, kind="ExternalOutput")
    if not fast:
        minwd = nc.dram_tensor("minw", [128, 2, NA2B, 8, W], BF16,
                               kind="ExternalOutput")

    units = _units()

    with tile.TileContext(nc) as tc:
        with (
            tc.tile_pool(name="wpool", bufs=1) as wp,
            tc.tile_pool(name="cpool", bufs=6) as cpp,
            tc.tile_pool(name="bpool", bufs=1) as bp,
            tc.tile_pool(name="tpool", bufs=2) as trp,
            tc.tile_pool(name="pspool", bufs=4, space="PSUM") as psp,
        ):
            ft0 = wp.tile([128, 2, 1024], F8, name="ft0")
            ft1 = wp.tile([128, 2, 1024], F8, name="ft1")
            fs0 = [wp.tile([128, 2, 512], F8, name=f"fs0{c}") for c in range(2)]
            fs1 = wp.tile([128, 2, 1024], F8, name="fs1")
            nc.sync.dma_start(ft0[:, :, 0:512], ft0ad[:, :, :])
            nc.sync.dma_start(fs0[0][:, :, :], fs0ad[:, :, :])
            nc.scalar.dma_start(fs0[1][:, :, :], fs0bd[:, :, :])
            nc.sync.dma_start(ft0[:, :, 512:1024], ft0bd[:, :, :])
            nc.scalar.dma_start(ft1[:, :, :], ft1d[:, :, :])
            nc.sync.dma_start(fs1[:, :, :], fs1d[:, :, :])

            # dummy copy: pulls the Act function table load into the DMA
            # dead time instead of delaying the first real PSUM copy
            dmy = bp.tile([128, 2], BF16, name="dmy")
            nc.vector.memset(dmy[:, 0:1], 0.0)
            nc.scalar.copy(dmy[:, 1:2], dmy[:, 0:1])

            stat = bp.tile([128, 4, KLOC], BF16, name="stat")
            statw = bp.tile([128, 2, NA2B, 8, W], BF16, name="statw")
            minstat = bp.tile([128, 2, 8] if fast else [128, 4, KLOC],
                              BF16, name="minstat")
            if not fast:
                minw = bp.tile([128, 2, NA2B, 8, W], BF16, name="minw")


            def rhs(h, m):
                if h == 0:
                    return fs0[m][:, :, :]
                return fs1[:, :, m * 512:(m + 1) * 512]

            for kind, (bt, h) in units:
                ps = psp.tile([128, 1024], F32, name=f"ps{bt}_{h}", tag="ps")
                ftc = ft0 if bt < 8 else ft1
                bl = (bt % 8) * 128
                for m in range(2):
                    nc.tensor.matmul(
                        ps[:, m * 512:(m + 1) * 512],
                        ftc[:, :, bl:bl + 128],
                        rhs(h, m),
                        start=True, stop=True, perf_mode=DRMODE,
                    )
                ps3 = ps.rearrange("p (k g) -> p k g", g=128)
                ksl = slice(h * 8, (h + 1) * 8)
                if kind == "D":
                    nc.vector.tensor_reduce(stat[:, bt, ksl], ps3[:],
                                            axis=AX.X, op=ALU.max)
                    if fast:
                        if (bt, h) in ((0, 0), (1, 1)):
                            nc.vector.tensor_reduce(minstat[:, bt, :],
                                                    ps3[:], axis=AX.X,
                                                    op=ALU.min)
                    else:
                        nc.vector.tensor_reduce(minstat[:, bt, ksl], ps3[:],
                                                axis=AX.X, op=ALU.min)
                else:
                    cp = cpp.tile([128, 8, 128], BF16, name=f"cp{bt}_{h}",
                                  tag="cp")
                    nc.scalar.copy(cp[:, :, :], ps[:, :])
                    for (ch, cs, cn) in CELLS:
                        if ch == h and cs <= bt < cs + cn:
                            break
                    l1o = trp.tile([128, 8, 64], BF16, name=f"l1o{bt}_{h}",
                                   tag="l1o", bufs=4)
                    nc.vector.tensor_tensor(l1o[:, :, :],
                                            cp[:, :, 0:64], cp[:, :, 64:128],
                                            ALU.max)
                    nc.vector.tensor_tensor(statw[:, h, bt - 4, :, :],
                                            l1o[:, :, 0:32], l1o[:, :, 32:64],
                                            ALU.max)
                    if not fast:
                        l1n = trp.tile([128, 8, 64], BF16, name=f"l1n{bt}_{h}",
                                       tag="l1n", bufs=4)
                        nc.vector.tensor_tensor(l1n[:, :, :],
                                                cp[:, :, 0:64],
                                                cp[:, :, 64:128], ALU.min)
                        nc.vector.tensor_tensor(minw[:, h, bt - 4, :, :],
                                                l1n[:, :, 0:32],
                                                l1n[:, :, 32:64], ALU.min)
                    for (ch, cs, cn) in CELLS:
                        if ch == h and bt == cs + cn - 1:
                            wsl = slice(cs - 4, cs - 4 + cn)
                            nc.sync.dma_start(statwd[:, ch, wsl, :, :],
                                              statw[:, ch, wsl, :, :])
                            if not fast:
                                nc.scalar.dma_start(minwd[:, ch, wsl, :, :],
                                                    minw[:, ch, wsl, :, :])

            nc.sync.dma_start(statd[:, :, :], stat[:, :, :])
            nc.scalar.dma_start(mind[:, :] if fast else mind[:, :, :],
                              minstat[:, :] if fast else minstat[:, :, :])

    nc.compile()
    _PROG_CACHE[fast] = nc
    return nc


def _quant(x):
    return np.clip(x * SCALE, -240.0, 240.0).astype(ml_dtypes.float8_e4m3fn)


def kernel(feats, feats_s, labels, labels_s, topk, num_instances):
    global LAST_RESULT
    feats = np.asarray(feats, dtype=np.float32)
    feats_s = np.asarray(feats_s, dtype=np.float32)
    labels = np.asarray(labels).astype(np.int64).ravel()
    labels_s = np.asarray(labels_s).astype(np.int64).ravel()
    tk, ni = int(topk), int(num_instances)
    assert feats.shape == (B, C), feats.shape
    assert tk * ni == G and feats_s.shape == (B, tk, C)

    Fs = feats_s.reshape(-1, C)                       # [16384, 256]
    glab = labels_s.reshape(KTOT, G)[:, 0]            # label of each block

    fast = bool(np.array_equal(labels_s, np.repeat(labels, tk)))
    if fast:
        for j in range(NCORES):
            own = np.isin(labels, glab[j * KLOC:(j + 1) * KLOC])
            want = np.zeros(B, dtype=bool)
            want[j * (B // NCORES):(j + 1) * (B // NCORES)] = True
            if not np.array_equal(own, want):
                fast = False
                break

    nc = _build(fast)

    in_maps = []
    for j in range(NCORES):
        shift = (B // NCORES) * j
        f_loc = np.roll(feats, -shift, axis=0) if fast else feats
        ftT = np.ascontiguousarray(
            f_loc.T.reshape(2, 128, B).transpose(1, 0, 2))
        fsT = Fs[j * MLOC:(j + 1) * MLOC].T.reshape(2, 128, MLOC)
        fsT = np.ascontiguousarray(fsT.transpose(1, 0, 2))   # [kp, kt, n]
        in_maps.append({
            "ftq0a": _quant(ftT[:, :, 0:512]),
            "ftq0b": _quant(ftT[:, :, 512:1024]),
            "ftq1": _quant(ftT[:, :, 1024:2048]),
            "fsq0a": _quant(fsT[:, :, 0:512]),
            "fsq0b": _quant(fsT[:, :, 512:1024]),
            "fsq1": _quant(fsT[:, :, 1024:2048]),
        })

    LAST_RESULT = run_bass_kernel_spmd(nc, in_maps, core_ids=list(range(NCORES)))

    # ---- host gather/unshard: exp, masks, cross-core sum, -log mean ----
    inv = 1.0 / (TEMP * SCALE * SCALE)
    pos = np.zeros(B, dtype=np.float64)
    neg = np.zeros(B, dtype=np.float64)
    for j in range(NCORES):
        res = LAST_RESULT.results[j]
        gl_j = glab[j * KLOC:(j + 1) * KLOC]              # [16]
        s03 = np.asarray(res["stat"], dtype=np.float32)   # [128, 4, 16]
        # statw [p, h, i, g, w] -> max over w -> [p, bt, h*8+g]
        sw = np.asarray(res["statw"], dtype=np.float32).max(axis=-1)
        sw = sw.transpose(0, 2, 1, 3).reshape(128, NA2B, KLOC)
        s = np.concatenate([s03, sw], axis=1)             # [128, 16, 16]
        emax = np.exp(s.transpose(1, 0, 2).reshape(B, KLOC) * inv)
        lab_loc = np.roll(labels, -(B // NCORES) * j) if fast else labels
        gmask = lab_loc[:, None] == gl_j[None, :]         # [2048, 16]
        negj = np.where(gmask, 0.0, emax).sum(axis=1)
        mn = np.asarray(res["mins"], dtype=np.float32)
        posj = np.zeros(B, dtype=np.float64)
        if fast:
            emin = np.exp(mn * inv)                       # [128, 2, 8]
            for t in range(2):
                rows = slice(t * 128, (t + 1) * 128)
                gm = gmask[rows, t * 8:(t + 1) * 8]       # [128, 8]
                posj[rows] = np.where(gm, emin[:, t, :], 0.0).sum(axis=1)
        else:
            mw = np.asarray(res["minw"], dtype=np.float32).min(axis=-1)
            mw = mw.transpose(0, 2, 1, 3).reshape(128, NA2B, KLOC)
            m_all = np.concatenate([mn, mw], axis=1)      # [128, 16, 16]
            emin = np.exp(m_all.transpose(1, 0, 2).reshape(B, KLOC) * inv)
            posj = np.where(gmask, emin, 0.0).sum(axis=1)
        if fast:
            shift = (B // NCORES) * j
            negj = np.roll(negj, shift)
            posj = np.roll(posj, shift)
        pos += posj
        neg += negj
    loss_i = -np.log(pos / (pos + neg + EPS) + EPS)
    return np.float32(loss_i.mean())


# revision 23
# speedup vs baseline: 1.1617x; 1.1617x over previous
"""Trainium2 Bass kernel for nn_CriterionLP (LP contrastive criterion loss).

Reference computation (B=2048 anchors, M=16384 supports, C=256, K=128 label
groups of G=128 supports each):
    sim   = (feats @ Fs.T) / TEMP                  [B, M]
    E     = exp(sim) grouped into K blocks of G    [B, K, G]
    pos   = exp(min sim over own-label block)      (one block per row)
    neg   = sum over other blocks of exp(max sim over block)
    loss  = mean_b( -log(pos/(pos+neg+eps) + eps) )

v22 design (support-sharded, 8 cores, no on-device collective):
  - fp8 e4m3 DoubleRow matmuls: full C=256 contraction per instruction,
    ~250-400ns per [128, 512] output; half the input DMA of bf16.
    fp8 end-to-end rel err ~6e-4 against a 2e-2 tolerance.
  - work unit = half b-tile [128, 1024] PSUM (2 banks, 4-deep ring).
  - TRN2 engine rules: gpsimd has no PSUM access and no max; Act has no
    max; DVE reads at most ONE PSUM operand per op. Split:
      D-units (8, b-tiles 0-3): one DVE tensor_reduce straight off PSUM
              -> final [128, 8] block maxes (W=1). Three D-units are
              placed at the very end so the kernel tail is a cheap
              reduce instead of copy+TT-chain+DMA.
      A-units (24, b-tiles 4-15): Act copies the [128, 1024] tile to
              bf16 SBUF; DVE TT-maxes the block halves at 2x, then one
              more per-unit 2x TT level down to W=32; the HOST finishes
              max-of-32. (Per-unit beats cell-batched chains: no
              cell-completion sync, better overlap. W=64 direct-ship was
              ~7us slower: 3MB of output DMA interferes mid-kernel.)
    Balances Act (~27us) and DVE (~27us); PE (~16-26us) hides under.
  - min stats are needed only for own-label blocks; after the row
    rotation they live in D-units (0,h0) and (1,h1): two extra DVE
    min-reduces off the same PSUM.
  - first input chunks are split small (128KB) so the first matmul can
    start as soon as possible after the ~8.6us DMA queue bring-up.
  - outputs: stat [128,4,16] (W=1 rows 0-3), statw [128,2,12,8,32]
    (h-major so each chunk is contiguous; DMA'd per cell as it
    finishes), mins [128,2,8]. The HOST does exp, label masks, the
    cross-core sum and the -log mean (the gather/unshard step). No
    AllReduce, no cross-core barrier, cores fully decoupled.
"""

import numpy as np
import ml_dtypes

import concourse.bass as bass
import concourse.bacc as bacc
import concourse.tile as tile
import concourse.mybir as mybir
from concourse.bass_utils import run_bass_kernel_spmd

VERSION_TAG = "v22"

F32 = mybir.dt.float32
BF16 = mybir.dt.bfloat16
F8 = mybir.dt.float8e4
AX = mybir.AxisListType
ALU = mybir.AluOpType
DRMODE = mybir.MatmulPerfMode.DoubleRow

TEMP = 0.05
EPS = 1e-6
SCALE = 16.0                # fp8 quantization scale (scores come out *S^2)
B, C = 2048, 256
NCORES = 8
KTOT, G = 128, 128          # label groups, supports per group
MLOC = 2048                 # support rows per core
KLOC = KTOT // NCORES       # groups per core (16)
NBT = B // 128              # b tiles of 128 rows (16)
W = 32                      # shipped stat width for A rows
NA2B = NBT - 4              # A b-tiles (4..15)

# A chain cells: (h, first bt, n bts)
CELLS = [(0, 4, 8), (0, 12, 4), (1, 4, 8), (1, 12, 4)]

_PROG_CACHE = {}
LAST_RESULT = None          # BassKernelResults of the most recent run


def _units():
    """32 (kind, (bt, h)) units. D spread early + 3 at the end (cheap
    tail); min-bearing D-units (0,h0) and (1,h1) first."""
    d = [(0, 0), (1, 1), (0, 1), (1, 0), (2, 0), (2, 1), (3, 0), (3, 1)]
    a = [(bt, 0) for bt in range(4, 16)] + [(bt, 1) for bt in range(4, 16)]
    d_slots = {0, 4, 8, 12, 16, 20, 24, 28}
    order = []
    di, ai = 0, 0
    for i in range(32):
        if i in d_slots:
            order.append(("D", d[di])); di += 1
        else:
            order.append(("A", a[ai])); ai += 1
    return order


def _chain(nc, pool, src, out_ap, dims, w0, w1, op, tag):
    """[128, *dims, w0] bf16 -> [128, *dims, w1] via TT halving (2x)."""
    cur, w = src, w0
    while w > w1:
        hw = w // 2
        if hw == w1:
            nxt_ap = out_ap
        else:
            nxt = pool.tile([128, *dims, hw], BF16, name=f"c{hw}{tag}",
                            tag=f"c{hw}_{'_'.join(map(str, dims))}", bufs=2)
            nxt_ap = nxt[:]
        sel0 = (slice(None),) * (1 + len(dims)) + (slice(0, hw),)
        sel1 = (slice(None),) * (1 + len(dims)) + (slice(hw, w),)
        nc.vector.tensor_tensor(nxt_ap, cur[sel0], cur[sel1], op)
        if hw > w1:
            cur = nxt
        w = hw


def _build(fast):
    if fast in _PROG_CACHE:
        return _PROG_CACHE[fast]

    nc = bacc.Bacc("TRN2", target_bir_lowering=False, debug=False,
                   num_devices=NCORES)
    # small first chunks so the first unit's inputs land early
    ft0ad = nc.dram_tensor("ftq0a", [128, 2, 512], F8, kind="ExternalInput")
    ft0bd = nc.dram_tensor("ftq0b", [128, 2, 512], F8, kind="ExternalInput")
    ft1d = nc.dram_tensor("ftq1", [128, 2, 1024], F8, kind="ExternalInput")
    fs0ad = nc.dram_tensor("fsq0a", [128, 2, 512], F8, kind="ExternalInput")
    fs0bd = nc.dram_tensor("fsq0b", [128, 2, 512], F8, kind="ExternalInput")
    fs1d = nc.dram_tensor("fsq1", [128, 2, 1024], F8, kind="ExternalInput")
    statd = nc.dram_tensor("stat", [128, 4, KLOC], BF16,
                           kind="ExternalOutput")
    statwd = nc.dram_tensor("statw", [128, 2, NA2B, 8, W], BF16,
                            kind="ExternalOutput")
    mind = nc.dram_tensor("mins", [128, 2, 8] if fast else [128, 4, KLOC],
                          BF16, kind="ExternalOutput")
    if not fast:
        minwd = nc.dram_tensor("minw", [128, 2, NA2B, 8, W], BF16,
                               kind="ExternalOutput")

    units = _units()

    with tile.TileContext(nc) as tc:
        with (
            tc.tile_pool(name="wpool", bufs=1) as wp,
            tc.tile_pool(name="cpool", bufs=6) as cpp,
            tc.tile_pool(name="bpool", bufs=1) as bp,
            tc.tile_pool(name="tpool", bufs=2) as trp,
            tc.tile_pool(name="pspool", bufs=4, space="PSUM") as psp,
        ):
            ft0 = wp.tile([128, 2, 1024], F8, name="ft0")
            ft1 = wp.tile([128, 2, 1024], F8, name="ft1")
            fs0 = [wp.tile([128, 2, 512], F8, name=f"fs0{c}") for c in range(2)]
            fs1 = wp.tile([128, 2, 1024], F8, name="fs1")
            nc.sync.dma_start(ft0[:, :, 0:512], ft0ad[:, :, :])
            nc.sync.dma_start(fs0[0][:, :, :], fs0ad[:, :, :])
            nc.scalar.dma_start(fs0[1][:, :, :], fs0bd[:, :, :])
            nc.sync.dma_start(ft0[:, :, 512:1024], ft0bd[:, :, :])
            nc.scalar.dma_start(ft1[:, :, :], ft1d[:, :, :])
            nc.sync.dma_start(fs1[:, :, :], fs1d[:, :, :])

            # dummy copy: pulls the Act function table load into the DMA
            # dead time instead of delaying the first real PSUM copy
            dmy = bp.tile([128, 2], BF16, name="dmy")
            nc.vector.memset(dmy[:, 0:1], 0.0)
            nc.scalar.copy(dmy[:, 1:2], dmy[:, 0:1])

            stat = bp.tile([128, 4, KLOC], BF16, name="stat")
            statw = bp.tile([128, 2, NA2B, 8, W], BF16, name="statw")
            minstat = bp.tile([128, 2, 8] if fast else [128, 4, KLOC],
                              BF16, name="minstat")
            if not fast:
                minw = bp.tile([128, 2, NA2B, 8, W], BF16, name="minw")


            def rhs(h, m):
                if h == 0:
                    return fs0[m][:, :, :]
                return fs1[:, :, m * 512:(m + 1) * 512]

            for kind, (bt, h) in units:
                ps = psp.tile([128, 1024], F32, name=f"ps{bt}_{h}", tag="ps")
                ftc = ft0 if bt < 8 else ft1
                bl = (bt % 8) * 128
                for m in range(2):
                    nc.tensor.matmul(
                        ps[:, m * 512:(m + 1) * 512],
                        ftc[:, :, bl:bl + 128],
                        rhs(h, m),
                        start=True, stop=True, perf_mode=DRMODE,
                    )
                ps3 = ps.rearrange("p (k g) -> p k g", g=128)
                ksl = slice(h * 8, (h + 1) * 8)
                if kind == "D":
                    nc.vector.tensor_reduce(stat[:, bt, ksl], ps3[:],
                                            axis=AX.X, op=ALU.max)
                    if fast:
                        if (bt, h) in ((0, 0), (1, 1)):
                            nc.vector.tensor_reduce(minstat[:, bt, :],
                                                    ps3[:], axis=AX.X,
                                                    op=ALU.min)
                            if (bt, h) == (1, 1):
                                # mins complete mid-kernel: ship now, off
                                # the tail
                                nc.scalar.dma_start(mind[:, :],
                                                    minstat[:, :])
                    else:
                        nc.vector.tensor_reduce(minstat[:, bt, ksl], ps3[:],
                                                axis=AX.X, op=ALU.min)
                else:
                    cp = cpp.tile([128, 8, 128], BF16, name=f"cp{bt}_{h}",
                                  tag="cp")
                    nc.scalar.copy(cp[:, :, :], ps[:, :])
                    for (ch, cs, cn) in CELLS:
                        if ch == h and cs <= bt < cs + cn:
                            break
                    l1o = trp.tile([128, 8, 64], BF16, name=f"l1o{bt}_{h}",
                                   tag="l1o", bufs=4)
                    nc.vector.tensor_tensor(l1o[:, :, :],
                                            cp[:, :, 0:64], cp[:, :, 64:128],
                                            ALU.max)
                    nc.vector.tensor_tensor(statw[:, h, bt - 4, :, :],
                                            l1o[:, :, 0:32], l1o[:, :, 32:64],
                                            ALU.max)
                    if not fast:
                        l1n = trp.tile([128, 8, 64], BF16, name=f"l1n{bt}_{h}",
                                       tag="l1n", bufs=4)
                        nc.vector.tensor_tensor(l1n[:, :, :],
                                                cp[:, :, 0:64],
                                                cp[:, :, 64:128], ALU.min)
                        nc.vector.tensor_tensor(minw[:, h, bt - 4, :, :],
                                                l1n[:, :, 0:32],
                                                l1n[:, :, 32:64], ALU.min)
                    for (ch, cs, cn) in CELLS:
                        if ch == h and bt == cs + cn - 1:
                            wsl = slice(cs - 4, cs - 4 + cn)
                            nc.sync.dma_start(statwd[:, ch, wsl, :, :],
                                              statw[:, ch, wsl, :, :])
                            if not fast:
                                nc.scalar.dma_start(minwd[:, ch, wsl, :, :],
                                                    minw[:, ch, wsl, :, :])

            nc.sync.dma_start(statd[:, :, :], stat[:, :, :])
            if not fast:
                nc.scalar.dma_start(mind[:, :, :], minstat[:, :, :])

    nc.compile()
    _PROG_CACHE[fast] = nc
    return nc


def _quant(x):
    return np.clip(x * SCALE, -240.0, 240.0).astype(ml_dtypes.float8_e4m3fn)


def kernel(feats, feats_s, labels, labels_s, topk, num_instances):
    global LAST_RESULT
    feats = np.asarray(feats, dtype=np.float32)
    feats_s = np.asarray(feats_s, dtype=np.float32)
    labels = np.asarray(labels).astype(np.int64).ravel()
    labels_s = np.asarray(labels_s).astype(np.int64).ravel()
    tk, ni = int(topk), int(num_instances)
    assert feats.shape == (B, C), feats.shape
    assert tk * ni == G and feats_s.shape == (B, tk, C)

    Fs = feats_s.reshape(-1, C)                       # [16384, 256]
    glab = labels_s.reshape(KTOT, G)[:, 0]            # label of each block

    fast = bool(np.array_equal(labels_s, np.repeat(labels, tk)))
    if fast:
        for j in range(NCORES):
            own = np.isin(labels, glab[j * KLOC:(j + 1) * KLOC])
            want = np.zeros(B, dtype=bool)
            want[j * (B // NCORES):(j + 1) * (B // NCORES)] = True
            if not np.array_equal(own, want):
                fast = False
                break

    nc = _build(fast)

    in_maps = []
    for j in range(NCORES):
        shift = (B // NCORES) * j
        f_loc = np.roll(feats, -shift, axis=0) if fast else feats
        ftT = np.ascontiguousarray(
            f_loc.T.reshape(2, 128, B).transpose(1, 0, 2))
        fsT = Fs[j * MLOC:(j + 1) * MLOC].T.reshape(2, 128, MLOC)
        fsT = np.ascontiguousarray(fsT.transpose(1, 0, 2))   # [kp, kt, n]
        in_maps.append({
            "ftq0a": _quant(ftT[:, :, 0:512]),
            "ftq0b": _quant(ftT[:, :, 512:1024]),
            "ftq1": _quant(ftT[:, :, 1024:2048]),
            "fsq0a": _quant(fsT[:, :, 0:512]),
            "fsq0b": _quant(fsT[:, :, 512:1024]),
            "fsq1": _quant(fsT[:, :, 1024:2048]),
        })

    LAST_RESULT = run_bass_kernel_spmd(nc, in_maps, core_ids=list(range(NCORES)))

    # ---- host gather/unshard: exp, masks, cross-core sum, -log mean ----
    inv = 1.0 / (TEMP * SCALE * SCALE)
    pos = np.zeros(B, dtype=np.float64)
    neg = np.zeros(B, dtype=np.float64)
    for j in range(NCORES):
        res = LAST_RESULT.results[j]
        gl_j = glab[j * KLOC:(j + 1) * KLOC]              # [16]
        s03 = np.asarray(res["stat"], dtype=np.float32)   # [128, 4, 16]
        # statw [p, h, i, g, w] -> max over w -> [p, bt, h*8+g]
        sw = np.asarray(res["statw"], dtype=np.float32).max(axis=-1)
        sw = sw.transpose(0, 2, 1, 3).reshape(128, NA2B, KLOC)
        s = np.concatenate([s03, sw], axis=1)             # [128, 16, 16]
        emax = np.exp(s.transpose(1, 0, 2).reshape(B, KLOC) * inv)
        lab_loc = np.roll(labels, -(B // NCORES) * j) if fast else labels
        gmask = lab_loc[:, None] == gl_j[None, :]         # [2048, 16]
        negj = np.where(gmask, 0.0, emax).sum(axis=1)
        mn = np.asarray(res["mins"], dtype=np.float32)
        posj = np.zeros(B, dtype=np.float64)
        if fast:
            emin = np.exp(mn * inv)                       # [128, 2, 8]
            for t in range(2):
                rows = slice(t * 128, (t + 1) * 128)
                gm = gmask[rows, t * 8:(t + 1) * 8]       # [128, 8]
                posj[rows] = np.where(gm, emin[:, t, :], 0.0).sum(axis=1)
        else:
            mw = np.asarray(res["minw"], dtype=np.float32).min(axis=-1)
            mw = mw.transpose(0, 2, 1, 3).reshape(128, NA2B, KLOC)
            m_all = np.concatenate([mn, mw], axis=1)      # [128, 16, 16]
            emin = np.exp(m_all.transpose(1, 0, 2).reshape(B, KLOC) * inv)
            posj = np.where(gmask, emin, 0.0).sum(axis=1)
        if fast:
            shift = (B // NCORES) * j
            negj = np.roll(negj, shift)
            posj = np.roll(posj, shift)
        pos += posj
        neg += negj
    loss_i = -np.log(pos / (pos + neg + EPS) + EPS)
    return np.float32(loss_i.mean())
